# revision 3
# baseline (speedup 1.0000x reference)
"""HBitLinear Trainium2 kernel (v2, transposed pipeline).

out = quant4(x @ H_1024) @ ternary(W).T, x:[8,8192,1024] f32, W:[1024,1024] f32.

Strategy (8 NeuronCores, data-parallel over the batch dim):
  - Host prep: x is cast fp16 and pre-TRANSPOSED to strips xT[g*128+i2, s]
    (halves input DMA and kills all on-device x transposes); W is ternarized
    on the host into fp8 ternT[j2, j1, o] plus fp16 row scales; H_1024 is
    split H_8 (x) H_128 (Sylvester).
  - Per 1024-token block: FHT8 butterflies across the 8 strips, all-SBUF
    fp16 on the vector engine (2x/4x DVE perf modes).
  - Per 128-token tile: M1 = ONE wide matmul pair with H128/32 fp16 as the
    stationary operand and the butterflied strips moving -> xh arrives
    TRANSPOSED [j2, chunk, s] in PSUM; fp32-exact (fp16 x +/-1/32 products
    accumulate exactly).
  - Quant stays in the transposed layout so M2 needs NO transposes at all:
    per-token amax via a gpsimd abs-max tree + one small PE transpose;
    reciprocal scale is broadcast along the free dim with a stride-0 DMA
    round-trip through DRAM; RNE via the fp32 magic-number trick fused into
    two DVE ops; q stored as fp8 (ints in [-7,7], exact).
  - M2: fp8 DoubleRow matmuls against ternT (exact integer arithmetic).
  - Epilogue: scalar engine applies the per-token scale, DVE applies the
    per-feature weight scale, output DMA'd as bf16 (halves output traffic);
    host upcasts to f32.
"""

import numpy as np

_CACHE: dict = {}

P = 128          # partitions
ST = 64          # token tiles per core (8192 / 128)
NCHUNK = 8       # 1024 / 128
NBLK = 8         # butterfly blocks (1024 tokens each)
SBLK = 1024      # tokens per block
MAGIC = float(np.float32(3 * 2 ** 22))  # 1.5*2^23: fp32 RNE rounding constant


def _sylvester(k: int) -> np.ndarray:
    h = np.array([[1]], dtype=np.int64)
    for _ in range(k):
        h = np.block([[h, h], [h, -h]])
    return h


def _build():
    import concourse.bass as bass
    import concourse.mybir as mybir
    import concourse.tile as tile
    from concourse import bacc
    from concourse.masks import make_identity

    dt = mybir.dt
    ALU = mybir.AluOpType
    ACTF = mybir.ActivationFunctionType

    nc = bacc.Bacc("TRN2", target_bir_lowering=False, debug=False)

    xt = nc.dram_tensor("xt", [NCHUNK * P, ST * P], dt.float16, kind="ExternalInput")
    tt = nc.dram_tensor("tt", [P, NCHUNK * P * NCHUNK], dt.float8e4, kind="ExternalInput")
    ws = nc.dram_tensor("ws", [NCHUNK * P], dt.float16, kind="ExternalInput")
    hm = nc.dram_tensor("hm", [P, P], dt.float16, kind="ExternalInput")
    out = nc.dram_tensor("out", [ST * P, NCHUNK * P], dt.bfloat16, kind="ExternalOutput")
    rsc_dram = nc.dram_tensor("rsc_scratch", [ST * P], dt.float32)

    from contextlib import ExitStack

    with tile.TileContext(nc) as tc, ExitStack() as stack:
        # ---------------- persistent constants ----------------
        const = stack.enter_context(tc.tile_pool(name="const", bufs=1))
        hm_sb = const.tile([P, P], dt.float16, tag="hm")
        nc.sync.dma_start(hm_sb[:], hm[:])
        tt_sb = const.tile([P, NCHUNK, NCHUNK * P], dt.float8e4, tag="tt")
        nc.sync.dma_start(tt_sb[:], tt[:].rearrange("p (a o) -> p a o", a=NCHUNK))
        id32 = const.tile([P, P], dt.float32, tag="id32")
        make_identity(nc, id32[:])
        magic_sb = const.tile([P, 1], dt.float32, tag="magic")
        nc.vector.memset(magic_sb[:], MAGIC)
        # wsb: fp16 weight scales broadcast to all 128 partitions via a
        # partition-stride-0 DMA read from DRAM.
        wsb = const.tile([P, NCHUNK * P], dt.float16, tag="wsb")
        ws_bcast = bass.AP(
            tensor=ws[:].tensor, offset=0, ap=[[0, P]] + list(ws[:].ap)
        )
        nc.gpsimd.dma_start(out=wsb[:], in_=ws_bcast)

        # ---------------- pools ----------------
        xpool = stack.enter_context(tc.tile_pool(name="xin", bufs=2))
        v1p = stack.enter_context(tc.tile_pool(name="v1", bufs=1))
        v2p = stack.enter_context(tc.tile_pool(name="v2", bufs=1))
        v3p = stack.enter_context(tc.tile_pool(name="v3", bufs=2))
        xhp = stack.enter_context(tc.tile_pool(name="xhsb", bufs=2))
        treep = stack.enter_context(tc.tile_pool(name="tree", bufs=2))
        scp = stack.enter_context(tc.tile_pool(name="scales", bufs=3))
        rbp = stack.enter_context(tc.tile_pool(name="rscb", bufs=3))
        qp = stack.enter_context(tc.tile_pool(name="q", bufs=2))
        op16 = stack.enter_context(tc.tile_pool(name="o16", bufs=2))
        obp = stack.enter_context(tc.tile_pool(name="obf", bufs=3))
        ps_xh = stack.enter_context(tc.tile_pool(name="ps_xh", bufs=2, space="PSUM"))
        ps_mt = stack.enter_context(tc.tile_pool(name="ps_mt", bufs=2, space="PSUM"))
        ps_g = stack.enter_context(tc.tile_pool(name="ps_g", bufs=2, space="PSUM"))

        for blk in range(NBLK):
            c0 = blk * SBLK
            # strips [i2, g, s] fp16 for 1024 tokens
            xb = xpool.tile([P, 2, 2, 2, SBLK], dt.float16, tag="xb")
            nc.sync.dma_start(
                xb[:],
                xt[:, c0 : c0 + SBLK].rearrange("(a p) s -> p a s", p=P),
            )
            # FHT8 butterflies over the strip index (3 stages, all DVE,
            # all-SBUF fp16 => 2x/4x DVE perf modes).
            v1 = v1p.tile([P, 2, 2, 2, SBLK], dt.float16, tag="v1")
            v2 = v2p.tile([P, 2, 2, 2, SBLK], dt.float16, tag="v2")
            v3 = v3p.tile([P, 2, 2, 2, SBLK], dt.float16, tag="v3")
            nc.vector.tensor_add(v1[:, 0, :, :, :], xb[:, 0, :, :, :], xb[:, 1, :, :, :])
            nc.vector.tensor_sub(v1[:, 1, :, :, :], xb[:, 0, :, :, :], xb[:, 1, :, :, :])
            nc.vector.tensor_add(v2[:, :, 0, :, :], v1[:, :, 0, :, :], v1[:, :, 1, :, :])
            nc.vector.tensor_sub(v2[:, :, 1, :, :], v1[:, :, 0, :, :], v1[:, :, 1, :, :])
            nc.vector.tensor_add(v3[:, :, :, 0, :], v2[:, :, :, 0, :], v2[:, :, :, 1, :])
            nc.vector.tensor_sub(v3[:, :, :, 1, :], v2[:, :, :, 0, :], v2[:, :, :, 1, :])
            v3f = bass.AP(
                tensor=v3[:].tensor,
                offset=v3[:].offset,
                ap=[list(v3[:].ap[0]), [SBLK, NCHUNK]] + [list(v3[:].ap[-1])],
            )  # view as [i2, j1, s]

            for t in range(NBLK):
                st = blk * NBLK + t
                s0 = st * P
                sl = t * P

                # M1: xh[j2, j1, s] = (H128/32)^T @ v3 -- H stationary, one
                # wide matmul per 4 chunks.
                xh = ps_xh.tile([P, NCHUNK, P], dt.float32, tag="xh")
                for h in range(2):
                    nc.tensor.matmul(
                        xh[:, 4 * h : 4 * h + 4, :],
                        hm_sb[:],
                        v3f[:, 4 * h : 4 * h + 4, sl : sl + P],
                        start=True, stop=True,
                    )
                # fp32 copy to SBUF (scalar) so DVE/gpsimd work runs at
                # SBUF rates and off the single-PSUM-operand limit.
                xh_sb = xhp.tile([P, NCHUNK, P], dt.float32, tag="xhsb")
                nc.scalar.copy(xh_sb[:], xh[:])

                # per-token absmax: chunk-dim abs-max via a reordered AP ...
                m1 = treep.tile([P, P], dt.float32, tag="m1")
                nc.vector.tensor_reduce(
                    m1[:], xh_sb[:].rearrange("p c s -> p s c"),
                    axis=mybir.AxisListType.X, op=ALU.max,
                    apply_absolute_value=True,
                )
                # ... then transpose [j2,s]->[s,j2] (PE) and reduce (DVE).
                m1t = ps_mt.tile([P, P], dt.float32, tag="m1t")
                nc.tensor.transpose(m1t[:], m1[:], id32[:])
                amax = scp.tile([P, 1], dt.float32, tag="amax")
                sc = scp.tile([P, 1], dt.float32, tag="sc")
                rsc = scp.tile([P, 1], dt.float32, tag="rsc")
                nc.vector.tensor_reduce(
                    amax[:], m1t[:], axis=mybir.AxisListType.X, op=ALU.max
                )
                nc.gpsimd.tensor_scalar(
                    sc[:], amax[:], 1e-5, float(np.float32(1.0 / 7.0)),
                    ALU.max, ALU.mult,
                )
                nc.vector.reciprocal(rsc[:], sc[:])

                # broadcast rsc along the free dim: DRAM round-trip with a
                # partition-stride-0 read.
                nc.sync.dma_start(
                    rsc_dram[s0 : s0 + P].rearrange("(a p) -> p a", p=P), rsc[:, :]
                )
                rsc_b = rbp.tile([P, P], dt.float32, tag="rscb")
                rsc_src = bass.AP(
                    tensor=rsc_dram[:].tensor, offset=s0, ap=[[0, P], [1, P]]
                )
                nc.gpsimd.dma_start(out=rsc_b[:], in_=rsc_src)

                # quantize: q = RNE(xh * rsc) as fp8 ints; magic trick fused
                # into one multiply + one scalar_tensor_tensor.
                tq = qp.tile([P, NCHUNK, P], dt.float32, tag="tq")
                q8 = qp.tile([P, NCHUNK, P], dt.float8e4, tag="q8")
                rsc_bc = bass.AP(
                    tensor=rsc_b[:].tensor, offset=rsc_b[:].offset,
                    ap=[list(rsc_b[:].ap[0]), [0, NCHUNK], list(rsc_b[:].ap[1])],
                )
                nc.vector.tensor_tensor(tq[:], xh_sb[:], rsc_bc, ALU.mult)
                magic_bc = bass.AP(
                    tensor=magic_sb[:].tensor, offset=magic_sb[:].offset,
                    ap=[list(magic_sb[:].ap[0]), [0, NCHUNK], [0, P]],
                )
                nc.vector.scalar_tensor_tensor(
                    q8[:], tq[:], MAGIC, magic_bc, ALU.add, ALU.subtract
                )

                # M2: G = q8^T . ternT (fp8 DoubleRow, exact ints), then
                # out = G * sc[token] * ws[feature].
                o16 = op16.tile([P, NCHUNK * P], dt.float16, tag="o16")
                for oh in range(2):
                    g = ps_g.tile([P, 512], dt.float32, tag="g")
                    for kk in range(NCHUNK // 2):
                        nc.tensor.matmul(
                            g[:], q8[:, 2 * kk : 2 * kk + 2, :],
                            tt_sb[:, 2 * kk : 2 * kk + 2, oh * 512 : (oh + 1) * 512],
                            start=(kk == 0), stop=(kk == NCHUNK // 2 - 1),
                            perf_mode=mybir.MatmulPerfMode.DoubleRow,
                        )
                    nc.scalar.activation(
                        o16[:, oh * 512 : (oh + 1) * 512], g[:], ACTF.Copy,
                        scale=sc[:],
                    )
                o = obp.tile([P, NCHUNK * P], dt.bfloat16, tag="o")
                nc.vector.tensor_tensor(o[:], o16[:], wsb[:], ALU.mult)
                nc.sync.dma_start(out[s0 : s0 + P, :], o[:])

    nc.finalize()
    return nc


def _get_nc():
    if "nc" not in _CACHE:
        _CACHE["nc"] = _build()
    return _CACHE["nc"]


def _prepare_inputs(x: np.ndarray, weight: np.ndarray) -> list[dict]:
    import ml_dtypes

    assert x.shape == (8, ST * P, NCHUNK * P) and x.dtype == np.float32
    assert weight.shape == (NCHUNK * P, NCHUNK * P)

    # ternary weight quantization on the host (fp32 math, exact vs reference)
    w = np.asarray(weight, dtype=np.float32)
    ws_f = np.maximum(np.abs(w).mean(axis=1, dtype=np.float64).astype(np.float32), np.float32(1e-5))
    n = w / ws_f[:, None]
    tern = (n > 0.5).astype(np.float32) - (n < -0.5).astype(np.float32)
    # ternT[j2, j1, o] = tern[o, j1*128 + j2], flattened [128, 8*1024] fp8
    ternT = np.ascontiguousarray(
        tern.T.reshape(NCHUNK, P, NCHUNK * P).transpose(1, 0, 2)
        .reshape(P, NCHUNK * NCHUNK * P)
    ).astype(ml_dtypes.float8_e4m3)
    ws16 = ws_f.astype(np.float16)
    hm16 = (_sylvester(7).astype(np.float32) / np.float32(32.0)).astype(np.float16)

    in_maps = []
    for i in range(8):
        xt = np.ascontiguousarray(x[i].astype(np.float16).T)  # [1024, 8192]
        in_maps.append({"xt": xt, "tt": ternT, "ws": ws16, "hm": hm16})
    return in_maps


def kernel(x: np.ndarray, weight: np.ndarray) -> np.ndarray:
    from concourse.bass_utils import run_bass_kernel_spmd

    nc = _get_nc()
    in_maps = _prepare_inputs(np.asarray(x), np.asarray(weight))
    res = run_bass_kernel_spmd(nc, in_maps, core_ids=list(range(8)))
    return np.stack(
        [res.results[i]["out"].astype(np.float32) for i in range(8)], axis=0
    )
